# revision 1
# baseline (speedup 1.0000x reference)
"""BatchAll triplet loss (multi-module variant) on 8 Trainium2 NeuronCores.

Math: labels = [0..191, 0..191] -- each anchor i has exactly ONE valid positive
j = (i+192) % 384, so the (i,j,k) cubic triplet tensor collapses to (i,k):

    loss_terms[i,k] = relu(d(i, p(i)) - d(i,k) + margin) * pm[i,k] * valid[i,k]
    out = sum(loss_terms) / (count(loss_terms > EPS) + EPS)

d(i,k) = sqrt(relu(2 + delta - 2*G[i,k]*rn_i*rn_k)) with raw Gram G and
rn = 1/||e||; the explicit relu guards the masked diagonal against the bf16
rounding of rn (delta=1e-5 keeps the exact-diagonal sqrt well-defined).

Precision: embeddings ship as fp8_e4m3 (PE Gram in fp8; norms computed from
the SAME fp8 values so the diagonal cancels exactly), rn / selector /
broadcast matmuls in bf16 (single-pass PE), weights bf16.  Measured rel-err
vs the fp32 reference ~1.5e-4 (gate is 2e-2).

Layout: [128, 192] -- partitions 0:48 = anchors x k-block 0 (local k 0..191),
64:112 = anchors x k-block 1; partitions 48:64 and 112:128 are pad driven by
16 junk lhsT columns and masked by pm=0.  Local column order per core:
[anchors | positives | rest], so the positive-pair values sit on the diagonal
of t2[0:48, 48:96] (and the self-pairs on diag of t2[0:48, 0:48], masked).

Structure notes (from NTFF traces):
- DMA completion semaphores lag the last descriptor by ~1us, and concurrent
  transfers interleave on the 16 shared DMA engines.  So the chain-head
  tensor (ER, which feeds norms -> rn -> broadcast -> everything) goes FIRST
  and ALONE on the sync ring with maximal 1.5KB/partition descriptors;
  et0/et1 queue behind it; pm rides the scalar ring (needed ~5us later).
- The ACT table loads (2x 1.3us, sqrt + square/sign sets) overlap the DMA
  phase and do not block the scalar sequencer's DMA issue.
- matmul operand partition bases must be 0/32/64 and lhsT/rhs bases must
  match; the rn row is built by three [128,1]->[1,128] transposes into one
  partition-0 [1,384] PSUM row.
- x1 = G * (-2 rn_a) runs on ACT (per-partition scale AP), freeing the DVE
  for the rn row copy that gates the RB broadcast panels.
- dpos^2 = 2+delta-2*chat_pos is produced in PSUM by pre-filling tp_ps with
  2+delta (rank-1 matmul, off the critical path) and accumulating the
  selector matmul on top; positive pairs are independent vectors so this
  column needs no relu guard.  Its small sqrt is ordered before the grid
  sqrt on ACT so dposm is ready when dms lands.
- count = sum(sign(lwpre - EPS)) accumulates on ACT in parallel with the
  DVE sum-reduce; the host maps sign-sum -> count = (ssum + cells)/2.
- fp8 on the DVE/ACT runs at 1 elem/cycle/lane (no small-dtype speedup),
  ~690ns per [128,512] chunk; norms are split DVE(c0,c2) / ACT(c1).
- engine register stores to the XLA output buffer fault at runtime (NEFF
  relocation does not cover InstSave addresses); the [1,2] result must go
  out via a regular DMA (~0.7us issue + ~0.6us latency).
- ~14us of the measured time is fixed NEFF scaffolding: ~6.5us entry
  (barriers, const memsets, engine config loads) and ~8.4us exit (queue
  flush + a 253-semaphore one-by-one reset sweep split across engines).
"""

import os
import sys

for _p in ("/opt/trn_rl_repo", "/root/.axon_site/_ro/trn_rl_repo"):
    if _p not in sys.path:
        sys.path.append(_p)

if "jax" not in sys.modules and os.environ.get("JAX_PLATFORMS") in ("cpu",):
    del os.environ["JAX_PLATFORMS"]

import ml_dtypes
import numpy as np

import concourse.bass as bass
import concourse.tile as tile
from concourse import mybir
from concourse.bacc import Bacc
from concourse.bass_utils import run_bass_kernel_spmd

F32 = mybir.dt.float32
BF16 = mybir.dt.bfloat16
F8 = mybir.dt.float8e4
U32 = mybir.dt.uint32
ALU = mybir.AluOpType
ACT = mybir.ActivationFunctionType

B = 192
N = 2 * B
D = 512
NCORES = 8
S = N // NCORES          # 48 anchors per core
MARGIN = 0.1
EPS = 1e-8
DELTA = 1e-5
CELLS = 128 * 192 * NCORES
N_WARMUP = 6


def build_nc() -> bass.Bass:
    nc = Bacc()

    er = nc.dram_tensor("er", [128, 1536], F8, kind="ExternalInput")
    et = nc.dram_tensor("et", [128, 1536], F8, kind="ExternalInput")
    pmw = nc.dram_tensor("pmw", [128, 192], BF16, kind="ExternalInput")
    out = nc.dram_tensor("out", [1, 2], F32, kind="ExternalOutput")

    with tile.TileContext(nc) as tc:
        with (
            tc.tile_pool(name="sb", bufs=1) as sb,
            tc.tile_pool(name="ps", bufs=1, space="PSUM") as ps,
        ):
            ER = sb.tile([128, 1536], F8, tag="ER")
            ET = sb.tile([128, 1536], F8, tag="ET")
            pm = sb.tile([128, 192], BF16, tag="pm")

            # ---- DMAs: all on ONE ring, strictly in need-order.  The
            #      chain-head tensor (ER) must run ALONE at full ring speed:
            #      concurrent transfers on the second ring interleave on the
            #      shared DMA engines and delay its completion semaphore.
            #      pm rides the scalar ring; it's needed ~5us later. ----
            nc.sync.dma_start(out=ER, in_=er[:, :])
            nc.sync.dma_start(out=ET, in_=et[:, :])
            nc.scalar.dma_start(out=pm, in_=pmw[:, :])

            # ---- identity via iota on the otherwise-idle gpsimd ----
            icol = sb.tile([128, 128], F32, tag="icol")
            nc.gpsimd.iota(icol, [[1, 128]], channel_multiplier=0,
                           allow_small_or_imprecise_dtypes=True)
            iprt = sb.tile([128, 1], F32, tag="iprt")
            nc.gpsimd.iota(iprt, [[0, 1]], channel_multiplier=1,
                           allow_small_or_imprecise_dtypes=True)
            identB = sb.tile([128, 128], BF16, tag="ident")
            nc.gpsimd.tensor_scalar(identB, icol, iprt, None, op0=ALU.is_equal)

            # ---- DVE constants ----
            wtile = sb.tile([128, 256], F8, tag="wtile")
            nc.vector.memset(wtile, 1.0)
            onesc = sb.tile([128, 1], BF16, tag="onesc")
            nc.vector.memset(onesc, 1.0)
            tdsrc = sb.tile([1, 1], F32, tag="tdsrc")
            nc.vector.memset(tdsrc, 1.0)
            beps = sb.tile([128, 1], F32, tag="beps")
            nc.vector.memset(beps, -EPS)
            ones1 = sb.tile([1, 128], BF16, tag="ones1")
            nc.vector.memset(ones1, 1.0)
            b2c = sb.tile([1, 1], BF16, tag="b2c")
            nc.vector.memset(b2c, 2.0 + DELTA)

            # ---- dummy sqrt pulls the ACT table early ----
            tdum = sb.tile([1, 1], F32, tag="tdum")
            nc.scalar.sqrt(tdum, tdsrc)

            # ---- PE warm-up bridging the DMA phase ----
            wps = ps.tile([128, 256], F32, tag="wps")
            for _ in range(N_WARMUP):
                nc.tensor.matmul(wps, wtile[:, 0:128], wtile,
                                 start=True, stop=True)

            # ---- norms from the fp8 rows: DVE chunks 0,2; ACT chunk 1 ----
            ns_col = sb.tile([128, 3], F32, tag="ns_col")
            junk = sb.tile([128, 512], BF16, tag="junk")
            junk2 = sb.tile([128, 512], BF16, tag="junk2")
            nc.vector.scalar_tensor_tensor(
                junk, ER[:, 0:512], 1.0, ER[:, 0:512], op0=ALU.mult,
                op1=ALU.mult, accum_out=ns_col[:, 0:1])
            nc.scalar.activation(junk2, ER[:, 512:1024], ACT.Square,
                                 accum_out=ns_col[:, 1:2])
            nc.vector.scalar_tensor_tensor(
                junk, ER[:, 1024:1536], 1.0, ER[:, 1024:1536], op0=ALU.mult,
                op1=ALU.mult, accum_out=ns_col[:, 2:3])

            # ---- selector tiles from ident: sel2[c,p]=1 iff p%64==c ----
            sel2 = sb.tile([48, 128], BF16, tag="sel2")
            nc.vector.memset(sel2, 0.0)
            nc.vector.tensor_copy(sel2[:, 0:48], identB[0:48, 0:48])
            nc.vector.tensor_copy(sel2[:, 64:112], identB[0:48, 0:48])
            sel2m = sb.tile([48, 128], BF16, tag="sel2m")
            nc.vector.memset(sel2m, 0.0)
            with nc.allow_low_precision("exact -2 in bf16"):
                nc.vector.tensor_scalar_mul(sel2m[:, 0:48],
                                            identB[0:48, 0:48], -2.0)
                nc.vector.tensor_scalar_mul(sel2m[:, 64:112],
                                            identB[0:48, 0:48], -2.0)

            # ---- Gram in [128,192] layout: 2 blocks x 4 chunks, fp8 ----
            g_ps = ps.tile([128, 192], F32, tag="G")
            for c in range(4):
                lhsT = ET[:, 384 * c:384 * c + 64]
                nc.tensor.matmul(g_ps[0:64, :], lhsT,
                                 ET[:, 384 * c:384 * c + 192],
                                 start=(c == 0), stop=(c == 3))
                nc.tensor.matmul(g_ps[64:128, :], lhsT,
                                 ET[:, 384 * c + 192:384 * c + 384],
                                 start=(c == 0), stop=(c == 3))

            # ---- prefill tp_ps = 2+delta (rank-1, off the critical path) ----
            tp_ps = ps.tile([128, 1], F32, tag="tp")
            nc.tensor.matmul(tp_ps, ones1, b2c, start=True, stop=False,
                             skip_group_check=True)

            # ---- rn = 1/sqrt(ns) in bf16 (relu-guarded downstream) ----
            nrm = sb.tile([128, 3], F32, tag="nrm")
            nc.scalar.sqrt(nrm, ns_col)
            rn_col = sb.tile([128, 3], BF16, tag="rn_col")
            with nc.allow_low_precision("bf16 rn; relu-guarded downstream"):
                nc.vector.reciprocal(rn_col, nrm)

            # ---- rnA[p] = -2*rn[p%64] via selector matmul ----
            rnA_ps = ps.tile([128, 1], F32, tag="rnA")
            nc.tensor.matmul(rnA_ps, sel2m, rn_col[0:48, 0:1],
                             start=True, stop=True)
            rnAsb = sb.tile([128, 1], F32, tag="rnAsb")
            nc.scalar.copy(rnAsb, rnA_ps)

            # ---- rn to one partition-0 row [1,384] via 3 transposes ----
            rts_ps = ps.tile([1, 384], BF16, tag="rnT")
            for j in range(3):
                nc.tensor.transpose(rts_ps[0:1, 128 * j:128 * (j + 1)],
                                    rn_col[:, j:j + 1], identB)
            rrow = sb.tile([1, 384], BF16, tag="rrow")
            with nc.allow_low_precision("bf16 rn rows; relu-guarded"):
                nc.vector.tensor_copy(rrow, rts_ps)

            # ---- RB[p,f] = rn_loc[192*(p//64) + f] via 4 rank-1 panels ----
            rb_ps = ps.tile([128, 192], F32, tag="RB")
            o64 = ones1[0:1, 0:64]
            nc.tensor.matmul(rb_ps[0:64, 0:128], o64, rrow[0:1, 0:128],
                             start=True, stop=True)
            nc.tensor.matmul(rb_ps[0:64, 128:192], o64, rrow[0:1, 128:192],
                             start=True, stop=True)
            nc.tensor.matmul(rb_ps[64:128, 0:64], o64, rrow[0:1, 192:256],
                             start=True, stop=True)
            nc.tensor.matmul(rb_ps[64:128, 64:192], o64, rrow[0:1, 256:384],
                             start=True, stop=True)

            # ---- t2 = -2 * G * rn_a * rn_k (x1 on ACT frees the DVE) ----
            x1 = sb.tile([128, 192], F32, tag="x1")
            nc.scalar.activation(x1, g_ps, ACT.Copy, bias=0.0, scale=rnAsb)
            t2_ps = ps.tile([128, 192], F32, tag="t2")
            nc.vector.tensor_mul(t2_ps, x1, rb_ps)

            # ---- positive-pair t2 values -> bf16 -> duplicated [128,1] ----
            tpj = sb.tile([48, 48], F32, tag="tpj")
            t2pos = sb.tile([48, 1], BF16, tag="t2pos")
            with nc.allow_low_precision("bf16 dpos path; |err| ~3e-3 abs"):
                nc.vector.scalar_tensor_tensor(
                    tpj, t2_ps[0:48, 48:96], 1.0, identB[0:48, 0:48],
                    op0=ALU.mult, op1=ALU.mult, accum_out=t2pos)
            # tp_ps is pre-filled with 2+delta by a rank-1 matmul (emitted
            # early, off the critical path), then the selector matmul
            # accumulates -2*chat_pos: tp_ps = dpos^2 directly.  Positive
            # pairs are independent vectors, so dpos^2 stays far from 0 and
            # needs no relu guard.
            nc.tensor.matmul(tp_ps, sel2, t2pos, start=False, stop=True)
            dpos = sb.tile([128, 1], F32, tag="dpos")
            nc.scalar.sqrt(dpos, tp_ps)

            # ---- d2 grid + sqrt (dpos's small sqrt is ordered first on ACT
            #      so dposm is ready before dms completes) ----
            d2r = sb.tile([128, 192], F32, tag="d2r")
            nc.vector.tensor_scalar(
                d2r, t2_ps, 2.0 + DELTA, 0.0, op0=ALU.add, op1=ALU.max)
            dms = sb.tile([128, 192], F32, tag="dms")
            nc.scalar.sqrt(dms, d2r)
            dposm = sb.tile([128, 1], F32, tag="dposm")
            nc.vector.tensor_scalar_add(dposm, dpos, MARGIN)

            # ---- weighted terms; sum on DVE, sign-count on ACT ----
            lwpre = sb.tile([128, 192], F32, tag="lwpre")
            nc.vector.scalar_tensor_tensor(
                lwpre, dms, dposm, pm, op0=ALU.subtract, op1=ALU.mult)
            stacked = sb.tile([128, 2], BF16, tag="stacked")
            lwj = sb.tile([128, 192], F32, tag="lwj")
            sgj = sb.tile([128, 192], F32, tag="sgj")
            with nc.allow_low_precision(
                    "bf16 partials: sign-sums are integers < 256 (exact); "
                    "lw-sums carry ~0.4% rounding, ~0.05% on the total"):
                nc.vector.tensor_scalar(
                    lwj, lwpre, 0.0, 0.0, op0=ALU.max, op1=ALU.add,
                    accum_out=stacked[:, 0:1])
                nc.scalar.activation(sgj, lwpre, ACT.Sign, bias=beps,
                                     scale=1.0, accum_out=stacked[:, 1:2])

            # ---- cross-partition reduce + writeback ----
            outp = ps.tile([1, 2], F32, tag="outp")
            nc.tensor.matmul(outp, onesc, stacked, start=True, stop=True)
            outs = sb.tile([1, 2], F32, tag="outs")
            nc.vector.tensor_copy(outs, outp)
            nc.sync.dma_start(out=out[:, :], in_=outs)

    nc.finalize()
    return nc


_NC_CACHE: dict = {}


def _get_nc() -> bass.Bass:
    if "nc" not in _NC_CACHE:
        _NC_CACHE["nc"] = build_nc()
    return _NC_CACHE["nc"]


def make_in_maps(output1, output2, weight):
    o1 = np.asarray(output1, dtype=np.float32)
    o2 = np.asarray(output2, dtype=np.float32)
    w = np.asarray(weight, dtype=np.float32)

    emb = np.concatenate([o1, o2], axis=0)
    w2 = np.tile(w, (2, 2))
    f8 = ml_dtypes.float8_e4m3
    a48 = np.arange(S)

    in_maps = []
    for c in range(NCORES):
        anchors = np.arange(c * S, c * S + S)
        pos = (anchors + B) % N
        used = np.zeros(N, dtype=bool)
        used[anchors] = True
        used[pos] = True
        loc = np.concatenate([anchors, pos, np.nonzero(~used)[0]])

        emb_loc = np.ascontiguousarray(emb[loc])
        embt = emb_loc.T
        ET = np.concatenate([embt[128 * k:128 * (k + 1), :] for k in range(4)],
                            axis=1).astype(f8)
        ER = np.concatenate([emb_loc[128 * t:128 * (t + 1), :] for t in range(3)],
                            axis=1).astype(f8)

        pmn = np.zeros((128, 192), dtype=np.float32)
        pmn[0:48, :] = -w2[anchors[:, None], loc[None, 0:192]]
        pmn[64:112, :] = -w2[anchors[:, None], loc[None, 192:384]]
        pmn[a48, a48] = 0.0          # k == i
        pmn[a48, S + a48] = 0.0      # k == p(i)

        in_maps.append({
            "er": ER,
            "et": ET,
            "pmw": pmn.astype(ml_dtypes.bfloat16),
        })
    return in_maps


def reduce_outputs(results):
    parts = np.stack([np.asarray(r["out"][0], dtype=np.float64)
                      for r in results])
    total = parts.sum(axis=0)
    count = (total[1] + CELLS) / 2.0
    return np.asarray(
        np.float32(total[0]) / (np.float32(count) + np.float32(EPS)),
        dtype=np.float32)


def kernel(output1, output2, weight):
    in_maps = make_in_maps(output1, output2, weight)
    res = run_bass_kernel_spmd(_get_nc(), in_maps, core_ids=list(range(NCORES)))
    return reduce_outputs(res.results)

